# revision 52
# baseline (speedup 1.0000x reference)
"""DSS layer kernel for Trainium2 (8 NeuronCores, SPMD, no collectives).

The conv kernel k[h,l] = Re(Wc @ exp(Lam*t)) has |exp(Lam*t)| = e^{-l/2}, so
taps beyond m=32 are < 1e-7 relative: the conv is a 33-tap causal FIR,
implemented as overlap-save block convolution with a half-shifted real DFT
(bins f+1/2: negacyclic conv, first K-1=32 outputs of each window aliased
and discarded). Window F=256 (128 complex bins), hop 224, 5 windows/core.

Device pipeline (per core = one batch element x one L-half, 1024 samples):
  - all inputs arrive as ONE host-prearranged dram blob; column-range DMAs
    ordered by first use flow on the sync queue (u ladder) and scalar queue
    (dfc A/B, khat, inverse tables) -- the DMA issue pipeline (~1.3us/DMA
    per queue) and the serialized DMA engines are the startup bottleneck.
  - u is fp16 and loaded ONCE; window starts are not 128-aligned, so the
    forward DFT is split at SBUF partition-quadrant boundaries into 2-4
    accumulating matmuls whose lhsT segments are host-built phase-shifted
    dfc slots (disjoint partition ranges packed into shared slots).
  - khat (input-dependent) is computed ON HOST in f64 and uploaded bf16.
  - spectrum product per window: ACT downcasts ui (PSUM->bf16), DVE does
    the ur copy + 3 muls + sub/add in bf16 (2x mode), GPSIMD one mul; the
    serial DVE chain paces the mid-pipeline.
  - inverse DFT in two h-tile halves per window ([128,2,256] PSUM, one
    bank) each followed by its gelu, so PSUM double-buffering fits in the
    8 banks alongside single-buffered forward tiles and 4 linear tiles.
  - final 512x512 linear in bf16, split into time chunks (448/448/128)
    emitted as their windows complete and interleaved ao-by-ao with the
    remaining inverse work; the last 128-wide chunk uses a single-bank
    [128,4,128] PSUM tile and (for the zero-bias case the harness always
    hits) one merged gelu, minimizing the post-last-window tail.
  - y2 stored fp16 per chunk on the sync queue; the final 128-wide chunk
    goes to its own contiguous [128,512] output tensor (full-bandwidth
    store on the tail); host reassembles and upcasts to f32.
A short warmup matmul chain pins the PE ramp anchor while DMAs land.
Sharding: 8 cores = (batch 4, L-half 2); each core owns all 512 channels
for its samples, so the final linear needs no cross-core communication.
"""

import numpy as np

H = 512
N = 64
B = 4
L = 2048
K = 33          # FIR taps
F = 256         # DFT window
HOP = 224
HALO = 32
NWIN = 5
LLOC = L // 2   # 1024 per core
ROWS = 1152     # 9 * 128 stored rows of u^T per core
NQ = 9
HT = H // 128   # 4 h-tiles
NCORES = 8
NWARM = 4

# forward-DFT lhsT slots, ordered by first window that needs them; each
# entry lists (dfc_row_lo, dfc_row_hi, base_partition) packed into one
# [128,128] slot (disjoint partition ranges share a slot)
_SLOTS = [
    [(0, 128, 0)],                             # 0: A
    [(128, 256, 0)],                           # 1: B
    [(192, 256, 0), (0, 32, 96)],              # 2: H @0:64, C @96:128
    [(32, 160, 0)],                            # 3: D
    [(160, 256, 0)],                           # 4: E (parts 96:128 zero)
    [(224, 256, 0), (0, 32, 32), (0, 64, 64)], # 5: K @0:32, I1 @32:64, F @64:128
    [(64, 192, 0)],                            # 6: G
    [(32, 96, 64)],                            # 7: I2 @64:128
    [(96, 224, 0)],                            # 8: J
]
NSEG = len(_SLOTS)
# per-window pieces: (u qcol, part_lo, part_hi, slot index)
# HW quadrant rule: base partition 0 -> <=128 rows, 32 -> <=32, 64 -> <=64
_PIECES = [
    [(0, 0, 128, 0), (1, 0, 128, 1)],
    [(1, 64, 128, 2), (2, 0, 128, 3), (3, 0, 96, 4)],
    [(3, 64, 128, 5), (4, 0, 128, 6), (5, 0, 64, 2)],
    [(5, 32, 64, 5), (5, 64, 128, 7), (6, 0, 128, 8), (7, 0, 32, 5)],
    [(7, 0, 128, 0), (8, 0, 128, 1)],
]

# blob column layout (u16 columns, host-prearranged)
_C_U01 = 0
_C_AB = 1024
_C_KH = 1536
_C_U23 = 2560
_C_U45 = 4352
_C_U67 = 5888
_C_U8 = 7424
_C_INV = 7936
_C_LWT = 8448
BLOBC = 10496
_UCOL = {0: 0, 1: 512, 2: 2560, 3: 3072, 4: 4352, 5: 4864,
         6: 5888, 7: 6400, 8: 7424}


def _scol(j):
    if j < 4:
        return _C_AB + j * 128
    if j < 10:
        return 3584 + (j - 4) * 128
    if j < 14:
        return 5376 + (j - 10) * 128
    return 6912 + (j - 14) * 128


_SCOL = [_scol(j) for j in range(2 * NSEG)]

_cache = {}


def _build_nc(zero_bias):
    import concourse.bacc as bacc
    import concourse.tile as tile
    from concourse import mybir
    from concourse.alu_op_type import AluOpType

    f32 = mybir.dt.float32
    bf16 = mybir.dt.bfloat16
    fp16 = mybir.dt.float16
    GELU = mybir.ActivationFunctionType.Gelu
    COPY = mybir.ActivationFunctionType.Copy
    u16 = mybir.dt.uint16

    nc = bacc.Bacc(None, target_bir_lowering=False)

    blob = nc.dram_tensor("blob", [128, BLOBC], u16, kind="ExternalInput")
    lb = nc.dram_tensor("lb", [128, HT], f32, kind="ExternalInput")
    y2 = nc.dram_tensor("y2", [H, LLOC], u16, kind="ExternalOutput")
    y2f = nc.dram_tensor("y2f", [128, 512], u16, kind="ExternalOutput")

    with tile.TileContext(nc) as tc:
        with (
            tc.tile_pool(name="consts", bufs=1) as consts,
            tc.tile_pool(name="scratch", bufs=3) as scratch,
        ):
            # ---------- loads ----------
            warm_sb = consts.tile([128, 256], bf16, tag="warm")
            nc.gpsimd.memset(warm_sb, 0.0)

            blob_sb = consts.tile([128, BLOBC], u16, tag="blob")
            lb_sb = consts.tile([128, HT], f32, tag="lb")

            # column-range loads from the host-prearranged blob, ordered by
            # first use; sync carries the ladder, scalar the two earliest
            # extras (its SEQ is needed for activations only after ~5us)
            def ld(eng, c0, c1):
                eng.dma_start(out=blob_sb[:, c0:c1], in_=blob[:, c0:c1])

            ld(nc.sync, _C_U01, _C_AB)       # u q0q1
            ld(nc.scalar, _C_AB, _C_KH)      # dfc slots A,B
            ld(nc.scalar, _C_KH, _C_U23)     # khat
            ld(nc.sync, _C_U23, _C_U45)      # u q2q3 + dfc slots for w1
            ld(nc.gpsimd, _C_INV, _C_LWT)    # inverse tables
            ld(nc.sync, _C_U45, _C_U67)      # u q4q5 + dfc slots for w2
            ld(nc.sync, _C_U67, _C_U8)       # u q6q7 + dfc slots for w3
            ld(nc.sync, _C_U8, _C_INV)       # u q8
            ld(nc.sync, _C_LWT, BLOBC)       # linear weights
            nc.sync.dma_start(out=lb_sb, in_=lb[:, :])

            def useg(q):
                c = _UCOL[q]
                return blob_sb[:, c:c + 512].bitcast(fp16)

            def dslot(s):
                c = _SCOL[s]
                return blob_sb[:, c:c + 128].bitcast(fp16)

            khr_sb = blob_sb[:, _C_KH:_C_KH + 512].bitcast(bf16)
            khi_sb = blob_sb[:, _C_KH + 512:_C_KH + 1024].bitcast(bf16)
            icc_sb = blob_sb[:, _C_INV:_C_INV + 256].bitcast(bf16)
            icsn_sb = blob_sb[:, _C_INV + 256:_C_INV + 512].bitcast(bf16)

            def lwseg(ai, o0, o1):
                c = _C_LWT + ai * 512
                return blob_sb[:, c + o0:c + o1].bitcast(bf16)

            y1_sb = consts.tile([128, HT, LLOC], bf16, tag="y1")
            y2_sb = consts.tile([128, HT, LLOC], fp16, tag="y2s")

            # ---------- pipeline ----------
            with (
                tc.tile_pool(name="ps_ur", bufs=1, space="PSUM") as ps_ur,
                tc.tile_pool(name="ps_ui", bufs=1, space="PSUM") as ps_ui,
                tc.tile_pool(name="ps_y1", bufs=2, space="PSUM") as ps_y1,
                tc.tile_pool(name="ps_lin", bufs=4, space="PSUM") as ps_lin,
            ):
                # preload both activation tables while DMAs are in flight so
                # no table load lands mid-pipeline
                pre_sb = scratch.tile([128, 2], bf16, tag="pre")
                nc.scalar.activation(out=pre_sb[:, 0:1], in_=warm_sb[:, 0:1],
                                     func=COPY)
                nc.scalar.activation(out=pre_sb[:, 1:2], in_=warm_sb[:, 0:1],
                                     func=GELU)

                # PE clock warmup: long accumulation chain on a zero tile
                wm_ps = ps_y1.tile([128, 2, 256], f32, tag="y1ps", name="wm_ps")
                for w in range(NWARM):
                    nc.tensor.matmul(wm_ps[:, 0, :224], lhsT=warm_sb[:, 0:128],
                                     rhs=warm_sb[:, :224],
                                     start=(w == 0), stop=(w == NWARM - 1))
                wm_out = scratch.tile([128, 1], f32, tag="wmout")
                nc.vector.tensor_copy(out=wm_out, in_=wm_ps[:, 0, 0:1])

                fwd_tiles = {}

                def emit_fwd(c):
                    pieces = _PIECES[c]
                    ur_ps = ps_ur.tile([128, H], f32, tag="ur", name=f"ur_{c}")
                    ui_ps = ps_ui.tile([128, H], f32, tag="ui", name=f"ui_{c}")
                    # ui group first and un-interleaved: its psum stops
                    # 2-3 matmuls earlier, so the uib->m2->pr pole of the
                    # product starts sooner each window
                    last = len(pieces) - 1
                    for i, (q, p0, p1, s) in enumerate(pieces):
                        rhs = useg(q)[p0:p1, :]
                        nc.tensor.matmul(ui_ps, lhsT=dslot(2 * s + 1)[p0:p1, :],
                                         rhs=rhs, start=(i == 0), stop=(i == last))
                    for i, (q, p0, p1, s) in enumerate(pieces):
                        rhs = useg(q)[p0:p1, :]
                        nc.tensor.matmul(ur_ps, lhsT=dslot(2 * s)[p0:p1, :],
                                         rhs=rhs, start=(i == 0), stop=(i == last))
                    fwd_tiles[c] = (ur_ps, ui_ps)

                prod_tiles = {}

                def emit_prod(c):
                    ur_ps, ui_ps = fwd_tiles.pop(c)
                    urb = scratch.tile([128, H], bf16, tag="urb", name=f"urb_{c}")
                    uib = scratch.tile([128, H], bf16, tag="uib", name=f"uib_{c}")
                    m1 = scratch.tile([128, H], bf16, tag="m1", name=f"m1_{c}")
                    m2 = scratch.tile([128, H], bf16, tag="m2", name=f"m2_{c}")
                    m3 = scratch.tile([128, H], bf16, tag="m3", name=f"m3_{c}")
                    m4 = scratch.tile([128, H], bf16, tag="m4", name=f"m4_{c}")
                    pr = scratch.tile([128, H], bf16, tag="pr", name=f"pr_{c}")
                    pi = scratch.tile([128, H], bf16, tag="pi", name=f"pi_{c}")
                    # ACT downcasts ui from PSUM; GPSIMD takes one mul;
                    # DVE handles the ur path, m4, and the final add/sub
                    nc.scalar.activation(out=uib, in_=ui_ps, func=COPY)
                    nc.vector.tensor_copy(out=urb, in_=ur_ps)
                    nc.gpsimd.tensor_mul(m2, uib, khi_sb)
                    nc.vector.tensor_mul(m1, urb, khr_sb)
                    nc.vector.tensor_mul(m3, urb, khi_sb)
                    nc.vector.tensor_mul(m4, uib, khr_sb)
                    nc.vector.tensor_sub(pr, m1, m2)
                    nc.vector.tensor_add(pi, m3, m4)
                    prod_tiles[c] = (pr, pi)

                def emit_inv(c):
                    pr, pi = prod_tiles.pop(c)
                    nt = min(HOP, LLOC - c * HOP)
                    for hh in range(2):
                        y1_ps = ps_y1.tile([128, 2, 256], f32, tag="y1ps",
                                           name=f"y1ps_{c}_{hh}")
                        for a in range(2):
                            at = 2 * hh + a
                            nc.tensor.matmul(y1_ps[:, a, :nt],
                                             lhsT=pr[:, at * 128:(at + 1) * 128],
                                             rhs=icc_sb[:, HALO:HALO + nt],
                                             start=True, stop=False)
                            nc.tensor.matmul(y1_ps[:, a, :nt],
                                             lhsT=pi[:, at * 128:(at + 1) * 128],
                                             rhs=icsn_sb[:, HALO:HALO + nt],
                                             start=False, stop=True)
                        nc.scalar.activation(
                            out=y1_sb[:, 2 * hh:2 * hh + 2,
                                      c * HOP:c * HOP + nt],
                            in_=y1_ps[:, :, :nt], func=GELU)

                def emit_lin(lo, hi, aos=(0, 1, 2, 3), store=True,
                             merged_gelu=False):
                    w = hi - lo
                    for ao in aos:
                        ps = ps_lin.tile([128, 512], f32, tag="linps",
                                         name=f"lin_{lo}_{ao}")
                        for ai in range(HT):
                            nc.tensor.matmul(
                                ps[:, :w],
                                lhsT=lwseg(ai, ao * 128, (ao + 1) * 128),
                                rhs=y1_sb[:, ai, lo:hi],
                                start=(ai == 0), stop=(ai == HT - 1))
                        nc.scalar.activation(out=y2_sb[:, ao, lo:hi],
                                             in_=ps[:, :w], func=GELU,
                                             bias=lb_sb[:, ao:ao + 1])
                    if store == "half":
                        a0, a1 = min(aos), max(aos) + 1
                        nc.sync.dma_start(
                            out=y2[a0 * 128:a1 * 128, lo:hi].bitcast(fp16)
                            .rearrange("(a p) t -> p a t", p=128),
                            in_=y2_sb[:, a0:a1, lo:hi])
                    elif store:
                        nc.sync.dma_start(
                            out=y2[:, lo:hi].bitcast(fp16)
                            .rearrange("(a p) t -> p a t", p=128),
                            in_=y2_sb[:, :, lo:hi])

                def emit_lin_final():
                    lo, hi = 896, 1024
                    w = hi - lo
                    # all four ao tiles in ONE psum bank (512B-aligned slices)
                    ps = ps_lin.tile([128, HT, 128], f32, tag="linps",
                                     name="lin_fin")
                    for ao in range(HT):
                        for ai in range(HT):
                            nc.tensor.matmul(
                                ps[:, ao, :w],
                                lhsT=lwseg(ai, ao * 128, (ao + 1) * 128),
                                rhs=y1_sb[:, ai, lo:hi],
                                start=(ai == 0), stop=(ai == HT - 1))
                    # contiguous final-chunk tile -> full-bandwidth store
                    y2f_sb = consts.tile([128, 512], fp16, tag="y2f")
                    y2f_v = y2f_sb[:, :].rearrange("p (a t) -> p a t", a=HT)
                    if zero_bias:
                        nc.scalar.activation(out=y2f_v,
                                             in_=ps[:, :, :w], func=GELU)
                    else:
                        for ao in range(HT):
                            nc.scalar.activation(
                                out=y2f_sb[:, ao * 128:(ao + 1) * 128],
                                in_=ps[:, ao, :w], func=GELU,
                                bias=lb_sb[:, ao:ao + 1])
                    nc.sync.dma_start(out=y2f[:, :].bitcast(fp16),
                                      in_=y2f_sb)

                emit_fwd(0)
                emit_fwd(1)
                emit_prod(0)
                emit_inv(0)
                emit_fwd(2)
                emit_prod(1)
                emit_inv(1)
                emit_fwd(3)
                emit_prod(2)
                emit_fwd(4)
                emit_inv(2)
                emit_prod(3)
                emit_prod(4)
                emit_lin(0, 448, aos=(0, 1), store="half")
                emit_inv(3)
                emit_lin(0, 448, aos=(2, 3), store="half")
                emit_lin(448, 896, aos=(0, 1), store="half")
                emit_inv(4)
                emit_lin(448, 896, aos=(2, 3), store="half")
                emit_lin_final()

    nc.compile()
    return nc


def _to_bf16_bits(x):
    u = np.ascontiguousarray(x, dtype=np.float32).view(np.uint32)
    r = (u + 0x7FFF + ((u >> 16) & 1)) >> 16
    return r.astype(np.uint16)


def _to_fp16_bits(x):
    return np.ascontiguousarray(x, dtype=np.float16).view(np.uint16)


def _build_tables(frequencies, decays, W, lin_w, lin_b):
    lam_re = (-np.exp(decays.astype(np.float32))).astype(np.float32)
    m = np.arange(K, dtype=np.float32)
    # match the reference's fp32 rounding of Lam[:,None] * t
    re = (lam_re[:, None] * m[None, :]).astype(np.float32).astype(np.float64)
    im = (frequencies.astype(np.float32)[:, None] * m[None, :]
          ).astype(np.float32).astype(np.float64)
    mag = np.exp(re)
    k = (W[..., 0].astype(np.float64) @ (mag * np.cos(im))
         - W[..., 1].astype(np.float64) @ (mag * np.sin(im)))  # (H, K)

    fb = np.arange(F // 2, dtype=np.float64) + 0.5
    tt = np.arange(F, dtype=np.float64)
    ang = 2 * np.pi * np.outer(tt, fb) / F
    dfc = np.cos(ang)
    dfsn = -np.sin(ang)
    iang = 2 * np.pi * np.outer(fb, tt) / F
    icc = (2.0 / F) * np.cos(iang)
    icsn = -(2.0 / F) * np.sin(iang)

    khr = (k @ dfc[:K]).T          # (F/2, H)
    khi = (k @ dfsn[:K]).T

    blob = np.zeros((128, BLOBC), np.uint16)
    for s, parts in enumerate(_SLOTS):
        for (r0, r1, p0) in parts:
            n = r1 - r0
            blob[p0:p0 + n, _SCOL[2 * s]:_SCOL[2 * s] + 128] = \
                _to_fp16_bits(dfc[r0:r1])
            blob[p0:p0 + n, _SCOL[2 * s + 1]:_SCOL[2 * s + 1] + 128] = \
                _to_fp16_bits(dfsn[r0:r1])
    blob[:, _C_KH:_C_KH + 512] = _to_bf16_bits(khr)
    blob[:, _C_KH + 512:_C_KH + 1024] = _to_bf16_bits(khi)
    blob[:, _C_INV:_C_INV + 256] = _to_bf16_bits(icc)
    blob[:, _C_INV + 256:_C_INV + 512] = _to_bf16_bits(icsn)
    lwtb = _to_bf16_bits(lin_w.astype(np.float32).T)     # (ci, o)
    blob[:, _C_LWT:BLOBC] = \
        lwtb.reshape(HT, 128, H).transpose(1, 0, 2).reshape(128, HT * H)
    return {
        "blob": blob,
        "lb": np.ascontiguousarray(
            lin_b.astype(np.float32).reshape(HT, 128).T),
    }


def _make_inmaps(u, tables):
    in_maps = []
    base = tables["blob"]
    for b in range(B):
        for half in range(2):
            t0 = half * LLOC
            uT = np.zeros((ROWS, H), np.float16)
            a0 = t0 - HALO
            s0, s1 = max(a0, 0), min(a0 + ROWS, L)
            uT[s0 - a0:s1 - a0] = u[b, :, s0:s1].T.astype(np.float16)
            u9 = uT.view(np.uint16).reshape(NQ, 128, H)
            bb = base.copy()
            for q in range(NQ):
                bb[:, _UCOL[q]:_UCOL[q] + 512] = u9[q]
            in_maps.append({"blob": bb, "lb": tables["lb"]})
    return in_maps


def kernel(u, frequencies, decays, W, lin_w, lin_b):
    from concourse.bass_utils import run_bass_kernel_spmd

    u = np.asarray(u, dtype=np.float32)
    tables = _build_tables(np.asarray(frequencies), np.asarray(decays),
                           np.asarray(W), np.asarray(lin_w), np.asarray(lin_b))

    zb = not np.any(np.asarray(lin_b))
    key = f"nc{int(zb)}"
    if key not in _cache:
        _cache[key] = _build_nc(zb)
    nc = _cache[key]

    in_maps = _make_inmaps(u, tables)
    res = run_bass_kernel_spmd(nc, in_maps, core_ids=list(range(NCORES)))
    out = np.empty((B, H, L), np.float32)
    for i, r in enumerate(res.results):
        b, half = divmod(i, 2)
        yh = r["y2"].view(np.float16).astype(np.float32)
        yf = (r["y2f"].view(np.float16).astype(np.float32)
              .reshape(128, HT, 128).transpose(1, 0, 2).reshape(H, 128))
        yh[:, 896:1024] = yf
        out[b, :, half * LLOC:(half + 1) * LLOC] = yh
    return out
